# revision 5
# baseline (speedup 1.0000x reference)
"""Distributed Trainium2 kernel v2 for nn_DistPredictor (gnn_message_passing).

score[e] = || hp[src[e]] - hpH[dst[e]] ||^2
  hp  = h @ W_w.T + W_b ;  hpH = hp @ H

Math rewrite (bias folded into dst table):
  diff_e = S[src_e] - T[dst_e]
  S[u] = h_u @ W_w.T              (no bias)
  T[v] = h_v @ (W_w.T @ H) + gamma,  gamma = W_b @ H - W_b

Strategy (8 NeuronCores, SPMD single program):
  - Edge -> core by dst ownership (12500-node ranges).
  - dst side: T table for the LOCAL node range only (12544 rows) built in
    DRAM (98 matmuls + sigma-staged row-major writes); per-edge 256B
    dma_gather (the only SWDGE traffic: ~77k descriptors at ~2.5ns/desc,
    the critical path).
  - src side: NO gather. Host bin-packs unique src nodes into NWIN
    windows with <=128 EDGES per window. Device streams packed hT_c,
    one matmul per window builds S rows (node-major, JIT in SBUF), then
    a one-hot matmul (lhsT = iota==rval compare on DVE) replicates
    window rows into edge-major slots.
  - Per window (== one 128-slot gather segment):
    diff = MM2_psum - gathered_T; DVE square; DVE reduce -> score col.
  - Host reassembles scores via slot->edge map.
"""

import sys

if "/opt/trn_rl_repo" not in sys.path:
    sys.path.insert(0, "/opt/trn_rl_repo")

import numpy as np
import ml_dtypes

# ---------------- configuration (patchable for mini-sim) ----------------
D = 128
P_CORES = 8

N = 100000
E = 600000

OWN = N // P_CORES        # nodes owned per core (dst ranges)
S_FLUSH = 14              # node tiles per staging flush for dst table
LOC_TILES = 98            # 98*128 = 12544 padded local nodes
LOC_PAD = LOC_TILES * 128

NI = 2048                 # indices per dma_gather instruction
NGATH = 38                # gather instructions per core
SLOTS = NI * NGATH        # 77824 edge slots = NWIN*128
NWIN = SLOTS // 128       # 608 src windows
WB = 8                    # windows per psum/staging batch

BF16 = ml_dtypes.bfloat16

_PROG = {}


def _sigma(local_node):
    """chunk-local node id -> permuted table row (matches staging flush)."""
    t = local_node // 128
    p = local_node % 128
    g = t // S_FLUSH
    i = t % S_FLUSH
    return g * (128 * S_FLUSH) + p * S_FLUSH + i


def _pack16r(idx, cap):
    """Pack idx -> [128, cap//16] int16 (i -> [i%16, i//16]), replicated
    across the 8 16-partition groups."""
    s = cap // 16
    out = np.zeros((16, s), np.int16)
    ar = np.arange(cap)
    out[ar % 16, ar // 16] = idx.astype(np.int16)
    return np.tile(out, (8, 1))


def _patch_swdge_lane_pinning():
    """Pin DMASW sem lanes {2q, 2q+1} to SWDGE queue q (Tile's round-robin
    is queue-unaware; with >1 queues a lane can see two queues' completions,
    breaking Tile's FIFO wait assumption)."""
    from concourse import tile_sem_assignment as tsa
    from concourse import mybir
    from concourse.tile_scheduler import DMAInst

    if getattr(tsa, "_qpin_patched", False):
        return
    cls = tsa.TileClockTick
    orig = cls._assign_tick

    def patched(self, inst):
        qn = getattr(inst, "queue_num", None)
        if (
            isinstance(inst, DMAInst)
            and inst.engine == mybir.EngineType.Pool
            and qn is not None
        ):
            lane_map = self.__dict__.setdefault("_qpin_map", {})
            if inst.name not in lane_map:
                cnts = self.__dict__.setdefault("_qpin_cnt", {})
                c = cnts.get(qn, 0)
                lane_map[inst.name] = (2 * qn + (c % 2)) % 8
                cnts[qn] = c + 1
            self.next_sw_dma_idx = lane_map[inst.name]
        return orig(self, inst)

    cls._assign_tick = patched
    tsa._qpin_patched = True


def _build_program():
    import concourse.bass as bass
    import concourse.tile as tile
    from concourse import bacc, mybir
    from concourse.library_config import mlp
    from concourse.tile_rust import add_dep_helper

    _patch_swdge_lane_pinning()

    f32 = mybir.dt.float32
    bf16 = mybir.dt.bfloat16
    i16 = mybir.dt.int16

    nc = bacc.Bacc(
        "TRN2",
        target_bir_lowering=False,
        debug=False,
        num_devices=P_CORES,
        num_swdge_queues=4,
    )

    slots = SLOTS
    nwin = NWIN
    nseg = NI // 128

    hT_c = nc.dram_tensor("hT_c", [128, nwin * 128], bf16, kind="ExternalInput")
    hT_loc = nc.dram_tensor("hT_loc", [128, LOC_PAD], bf16, kind="ExternalInput")
    WT = nc.dram_tensor("WT", [128, 128], bf16, kind="ExternalInput")
    M2 = nc.dram_tensor("M2", [128, 128], bf16, kind="ExternalInput")
    gamma_bc = nc.dram_tensor("gamma_bc", [128, 128], bf16, kind="ExternalInput")
    negI = nc.dram_tensor("negI", [128, 128], bf16, kind="ExternalInput")
    fp8 = mybir.dt.float8e4
    onehot = nc.dram_tensor("onehot", [128, slots], fp8, kind="ExternalInput")
    didx = nc.dram_tensor("didx", [128, slots // 16], i16, kind="ExternalInput")
    score = nc.dram_tensor("score", [128, nwin], f32, kind="ExternalOutput")

    T_tab = nc.dram_tensor("T_tab", [LOC_PAD, 128], bf16)

    with tile.TileContext(nc) as tc:
        nc.gpsimd.load_library(mlp)
        with (
            tc.tile_pool(name="const", bufs=1) as cpool,
            tc.tile_pool(name="loc", bufs=2) as locpool,
            tc.tile_pool(name="psD", bufs=2, space="PSUM") as psD,
            tc.tile_pool(name="ht", bufs=16) as htpool,
            tc.tile_pool(name="win", bufs=4) as winpool,
            tc.tile_pool(name="oh", bufs=16) as ohpool,
            tc.tile_pool(name="ps2", bufs=3, space="PSUM") as ps2pool,
            tc.tile_pool(name="gat", bufs=12) as gpool,
            tc.tile_pool(name="ed", bufs=2) as epool,
            tc.tile_pool(name="idx", bufs=1) as ipool,
            tc.tile_pool(name="out", bufs=1) as opool,
        ):
            wt_t = cpool.tile([128, 128], bf16)
            nc.sync.dma_start(wt_t[:], WT[:])
            m2_t = cpool.tile([128, 128], bf16)
            nc.sync.dma_start(m2_t[:], M2[:])
            gm_t = cpool.tile([128, 128], bf16)
            nc.sync.dma_start(gm_t[:], gamma_bc[:])
            negI_t = cpool.tile([128, 128], bf16)
            nc.sync.dma_start(negI_t[:], negI[:])

            score_sb = opool.tile([128, nwin], f32)

            di_sb = ipool.tile([128, slots // 16], i16)
            for g in range(NGATH):
                c0 = g * (NI // 16)
                nc.sync.dma_start(di_sb[:, c0:c0 + NI // 16],
                                  didx[:, c0:c0 + NI // 16])

            # ---------- phase 1: local T table (this core's dst range) -------
            for fl in range(LOC_TILES // S_FLUSH):
                htl = locpool.tile([128, S_FLUSH * 128], bf16, tag="htl")
                nc.sync.dma_start(
                    htl[:], hT_loc[:, fl * S_FLUSH * 128:(fl + 1) * S_FLUSH * 128])
                stH = locpool.tile([128, S_FLUSH, 128], bf16, tag="stH")
                i = 0
                while i < S_FLUSH:
                    w = min(4, S_FLUSH - i)
                    psT = psD.tile([128, 4, 128], f32, tag="psT")
                    for u in range(w):
                        nc.tensor.matmul(
                            psT[:, u, :],
                            lhsT=htl[:, (i + u) * 128:(i + u + 1) * 128],
                            rhs=m2_t[:])
                    nc.vector.tensor_tensor(
                        out=stH[:, i:i + w, :], in0=psT[:, :w, :],
                        in1=gm_t[:].rearrange("p (a f) -> p a f", a=1)
                        .broadcast_to([128, w, 128]),
                        op=mybir.AluOpType.add,
                    )
                    i += w
                t_write = nc.sync.dma_start(
                    T_tab[fl * S_FLUSH * 128:(fl + 1) * S_FLUSH * 128, :]
                    .rearrange("(p i) f -> p i f", p=128),
                    stH[:],
                )

            # ---------- phase 2: dst gathers (the critical SWDGE pipe) ------
            ps1_ctx = tc.tile_pool(name="ps1", bufs=2, space="PSUM")
            ps2_ctx = tc.tile_pool(name="ps2", bufs=2, space="PSUM")
            ps1pool = ps1_ctx.__enter__()
            ps2pool = ps2_ctx.__enter__()
            gd_tiles = []
            for g in range(NGATH):
                gd = gpool.tile([128, nseg, 128], bf16, tag="gd")
                nc.gpsimd.dma_gather(
                    gd[:], T_tab[:], di_sb[:, g * (NI // 16):(g + 1) * (NI // 16)],
                    NI, NI, 128, single_packet=False, queue_num=g % 4,
                )
                gd_tiles.append(gd)

            # ---------- phase 3: src windows + edge math ---------------------
            diff_tiles = {}
            for wb in range(nwin // WB):
                w0 = wb * WB
                hts = htpool.tile([128, WB * 128], bf16, tag="hts")
                d1 = nc.sync.dma_start(hts[:], hT_c[:, w0 * 128:(w0 + WB) * 128])
                if wb < 6:
                    # keep phase-1 (T table) DMA unstarved: phase-3 streams
                    # start only after the last T_tab write is queued
                    add_dep_helper(d1.ins, t_write.ins,
                                   reason="phase3 streams after T table")
                ps1 = ps1pool.tile([128, WB, 128], f32, tag="ps1")
                for u in range(WB):
                    nc.tensor.matmul(
                        ps1[:, u, :],
                        lhsT=hts[:, u * 128:(u + 1) * 128],
                        rhs=wt_t[:],
                    )
                win = winpool.tile([128, WB, 128], bf16, tag="win")
                nc.scalar.activation(
                    win[:], ps1[:], func=mybir.ActivationFunctionType.Copy)

                oh = ohpool.tile([128, WB * 128], fp8, tag="oh")
                d2 = nc.sync.dma_start(
                    oh[:], onehot[:, w0 * 128:(w0 + WB) * 128])
                if wb < 6:
                    add_dep_helper(d2.ins, t_write.ins,
                                   reason="phase3 streams after T table")

                ps2 = ps2pool.tile([128, WB, 128], f32, tag="ps2")
                for u in range(WB):
                    nc.tensor.matmul(
                        ps2[:, u, :],
                        lhsT=oh[:, u * 128:(u + 1) * 128],
                        rhs=win[:, u, :],
                    )
                g = w0 // nseg
                s0 = w0 % nseg
                assert s0 + WB <= nseg
                if s0 == 0:
                    diff_w = epool.tile([128, nseg, 128], bf16, tag="diff")
                    diff_tiles[g] = diff_w
                diff_w = diff_tiles[g]
                nc.vector.tensor_tensor(
                    out=diff_w[:, s0:s0 + WB, :],
                    in0=gd_tiles[g][:, s0:s0 + WB, :], in1=ps2[:],
                    op=mybir.AluOpType.subtract,
                )
                if s0 + WB == nseg:
                    sq_w = epool.tile([128, nseg, 128], bf16, tag="sq")
                    if g % 4 != 3:
                        # plain ACT Square at copy rate (no accum read)
                        nc.scalar.activation(
                            sq_w[:], diff_w[:],
                            func=mybir.ActivationFunctionType.Square)
                    else:
                        nc.vector.tensor_tensor(
                            out=sq_w[:], in0=diff_w[:], in1=diff_w[:],
                            op=mybir.AluOpType.mult,
                        )
                    nc.vector.tensor_reduce(
                        out=score_sb[:, g * nseg:(g + 1) * nseg],
                        in_=sq_w[:],
                        axis=mybir.AxisListType.X,
                        op=mybir.AluOpType.add,
                    )

            nc.sync.dma_start(score[:], score_sb[:])
            ps2_ctx.__exit__(None, None, None)
            ps1_ctx.__exit__(None, None, None)

    nc.finalize()
    return nc


def _binpack(ku):
    """Next-fit decreasing: pack rows (ku[r] edges each) into windows with
    <=128 edges and <=128 rows. Returns (win_of_row, pos_of_row, nw)."""
    nu = len(ku)
    order = np.argsort(-ku, kind="stable")
    win_of_row = np.empty(nu, np.int64)
    pos_of_row = np.empty(nu, np.int64)
    w = 0
    edges_w = 0
    rows_w = 0
    for r in order:
        k = int(ku[r])
        if edges_w + k > 128 or rows_w >= 128:
            w += 1
            edges_w = 0
            rows_w = 0
        win_of_row[r] = w
        pos_of_row[r] = rows_w
        edges_w += k
        rows_w += 1
    return win_of_row, pos_of_row, w + 1


def _prep_inputs(h, src, dst, W_w, W_b, H):
    """Host-side: shard edges by dst owner, bin-pack src windows, build
    per-core input maps + slot->edge maps."""
    h = np.asarray(h, dtype=np.float32)
    src = np.asarray(src).astype(np.int64)
    dst = np.asarray(dst).astype(np.int64)
    W_w = np.asarray(W_w, dtype=np.float32)
    W_b = np.asarray(W_b, dtype=np.float32)
    H = np.asarray(H, dtype=np.float32)

    hT_full = h.T.astype(BF16)                              # [128, N]
    WT = np.ascontiguousarray(W_w.T).astype(BF16)           # S: lhsT=hT, rhs=WT
    M2 = (W_w.T @ H).astype(BF16)                           # T: lhsT=hT, rhs=M2
    gamma = (W_b @ H - W_b).astype(np.float32)
    gamma_bc = np.tile(gamma[None, :], (128, 1)).astype(BF16)
    negI_m = (-np.eye(128, dtype=np.float32)).astype(BF16)
    iota128 = np.arange(128)

    owner = np.minimum(dst // OWN, P_CORES - 1)

    in_maps = []
    slotmaps = []
    for c in range(P_CORES):
        sel = np.nonzero(owner == c)[0]
        ne = len(sel)
        if ne > SLOTS:
            raise RuntimeError(f"core {c}: {ne} edges > {SLOTS} slots")
        src_c = src[sel]
        dst_loc = dst[sel] - c * OWN

        uniq, inv = np.unique(src_c, return_inverse=True)
        ku = np.bincount(inv)
        win_of_row, pos_of_row, nw = _binpack(ku)
        if nw > NWIN:
            raise RuntimeError(f"core {c}: {nw} windows > {NWIN}")

        # slot assignment: edges of window w -> slots [128w, 128w + n_w)
        e_win = win_of_row[inv]
        counts = np.bincount(e_win, minlength=nw)
        if counts.max() > 128:
            raise RuntimeError(f"core {c}: window overflow {counts.max()}")
        order_e = np.argsort(e_win, kind="stable")
        starts = np.concatenate([[0], np.cumsum(counts)])
        slot_ids = np.empty(ne, np.int64)
        for w in range(nw):
            lo, hi = starts[w], starts[w + 1]
            slot_ids[order_e[lo:hi]] = 128 * w + np.arange(hi - lo)

        didx_full = np.zeros(SLOTS, np.int64)
        rvals_full = np.full(SLOTS, -1, np.int64)
        smap = np.full(SLOTS, -1, np.int64)
        didx_full[slot_ids] = _sigma(dst_loc)
        rvals_full[slot_ids] = pos_of_row[inv]
        smap[slot_ids] = sel
        onehot_full = (iota128[:, None] == rvals_full[None, :]).astype(
            ml_dtypes.float8_e4m3)

        # packed hT_c: (window w, pos p) -> uniq node column
        hT_win = np.zeros((128, NWIN * 128), dtype=BF16)
        cols = win_of_row * 128 + pos_of_row
        hT_win[:, cols] = hT_full[:, uniq]

        lo = c * OWN
        hT_loc = np.zeros((128, LOC_PAD), dtype=BF16)
        avail = min(N, lo + LOC_PAD) - lo
        hT_loc[:, :avail] = hT_full[:, lo:lo + avail]

        in_maps.append({
            "hT_c": hT_win,
            "hT_loc": hT_loc,
            "WT": WT,
            "M2": M2,
            "gamma_bc": gamma_bc,
            "negI": negI_m,
            "onehot": onehot_full,
            "didx": _pack16r(didx_full, SLOTS),
        })
        slotmaps.append(smap)
    return in_maps, slotmaps


def kernel(h, src, dst, W_w, W_b, H):
    from concourse.bass_utils import run_bass_kernel_spmd

    if "nc" not in _PROG:
        _PROG["nc"] = _build_program()
    nc = _PROG["nc"]

    in_maps, slotmaps = _prep_inputs(h, src, dst, W_w, W_b, H)
    res = run_bass_kernel_spmd(nc, in_maps, list(range(P_CORES)))

    out = np.zeros(E, np.float32)
    for c in range(P_CORES):
        dev = res.results[c]["score"]                       # [128, NWIN]
        padded = dev.T.ravel()                              # slot = 128*w + p
        smap = slotmaps[c]
        m = smap >= 0
        out[smap[m]] = padded[m]
    return out


# revision 6
# speedup vs baseline: 1.0108x; 1.0108x over previous
"""Distributed Trainium2 kernel v2 for nn_DistPredictor (gnn_message_passing).

score[e] = || hp[src[e]] - hpH[dst[e]] ||^2
  hp  = h @ W_w.T + W_b ;  hpH = hp @ H

Math rewrite (bias folded into dst table):
  diff_e = S[src_e] - T[dst_e]
  S[u] = h_u @ W_w.T              (no bias)
  T[v] = h_v @ (W_w.T @ H) + gamma,  gamma = W_b @ H - W_b

Strategy (8 NeuronCores, SPMD single program):
  - Edge -> core by dst ownership (12500-node ranges).
  - dst side: T table for the LOCAL node range only (12544 rows) built in
    DRAM (98 matmuls + sigma-staged row-major writes); per-edge 256B
    dma_gather (the only SWDGE traffic: ~77k descriptors at ~2.5ns/desc,
    the critical path).
  - src side: NO gather. Host bin-packs unique src nodes into NWIN
    windows with <=128 EDGES per window. Device streams packed hT_c,
    one matmul per window builds S rows (node-major, JIT in SBUF), then
    a one-hot matmul (lhsT = iota==rval compare on DVE) replicates
    window rows into edge-major slots.
  - Per window (== one 128-slot gather segment):
    diff = MM2_psum - gathered_T; DVE square; DVE reduce -> score col.
  - Host reassembles scores via slot->edge map.
"""

import sys

if "/opt/trn_rl_repo" not in sys.path:
    sys.path.insert(0, "/opt/trn_rl_repo")

import numpy as np
import ml_dtypes

# ---------------- configuration (patchable for mini-sim) ----------------
D = 128
P_CORES = 8

N = 100000
E = 600000

OWN = N // P_CORES        # nodes owned per core (dst ranges)
S_FLUSH = 14              # node tiles per staging flush for dst table
LOC_TILES = 98            # 98*128 = 12544 padded local nodes
LOC_PAD = LOC_TILES * 128

NI = 2048                 # indices per dma_gather instruction
NGATH = 38                # gather instructions per core
SLOTS = NI * NGATH        # 77824 edge slots = NWIN*128
NWIN = SLOTS // 128       # 608 src windows
WB = 8                    # windows per psum/staging batch

BF16 = ml_dtypes.bfloat16

_PROG = {}


def _sigma(local_node):
    """chunk-local node id -> permuted table row (matches staging flush)."""
    t = local_node // 128
    p = local_node % 128
    g = t // S_FLUSH
    i = t % S_FLUSH
    return g * (128 * S_FLUSH) + p * S_FLUSH + i


def _pack16r(idx, cap):
    """Pack idx -> [128, cap//16] int16 (i -> [i%16, i//16]), replicated
    across the 8 16-partition groups."""
    s = cap // 16
    out = np.zeros((16, s), np.int16)
    ar = np.arange(cap)
    out[ar % 16, ar // 16] = idx.astype(np.int16)
    return np.tile(out, (8, 1))


def _patch_swdge_lane_pinning():
    """Pin DMASW sem lanes {2q, 2q+1} to SWDGE queue q (Tile's round-robin
    is queue-unaware; with >1 queues a lane can see two queues' completions,
    breaking Tile's FIFO wait assumption)."""
    from concourse import tile_sem_assignment as tsa
    from concourse import mybir
    from concourse.tile_scheduler import DMAInst

    if getattr(tsa, "_qpin_patched", False):
        return
    cls = tsa.TileClockTick
    orig = cls._assign_tick

    def patched(self, inst):
        qn = getattr(inst, "queue_num", None)
        if (
            isinstance(inst, DMAInst)
            and inst.engine == mybir.EngineType.Pool
            and qn is not None
        ):
            lane_map = self.__dict__.setdefault("_qpin_map", {})
            if inst.name not in lane_map:
                cnts = self.__dict__.setdefault("_qpin_cnt", {})
                c = cnts.get(qn, 0)
                lane_map[inst.name] = (2 * qn + (c % 2)) % 8
                cnts[qn] = c + 1
            self.next_sw_dma_idx = lane_map[inst.name]
        return orig(self, inst)

    cls._assign_tick = patched
    tsa._qpin_patched = True


def _build_program():
    import concourse.bass as bass
    import concourse.tile as tile
    from concourse import bacc, mybir
    from concourse.library_config import mlp
    from concourse.tile_rust import add_dep_helper

    _patch_swdge_lane_pinning()

    f32 = mybir.dt.float32
    bf16 = mybir.dt.bfloat16
    i16 = mybir.dt.int16

    nc = bacc.Bacc(
        "TRN2",
        target_bir_lowering=False,
        debug=False,
        num_devices=P_CORES,
        num_swdge_queues=4,
    )

    slots = SLOTS
    nwin = NWIN
    nseg = NI // 128

    hT_c = nc.dram_tensor("hT_c", [128, nwin * 128], bf16, kind="ExternalInput")
    hT_loc = nc.dram_tensor("hT_loc", [128, LOC_PAD], bf16, kind="ExternalInput")
    WT = nc.dram_tensor("WT", [128, 128], bf16, kind="ExternalInput")
    M2 = nc.dram_tensor("M2", [128, 128], bf16, kind="ExternalInput")
    gamma_bc = nc.dram_tensor("gamma_bc", [128, 128], bf16, kind="ExternalInput")
    negI = nc.dram_tensor("negI", [128, 128], bf16, kind="ExternalInput")
    fp8 = mybir.dt.float8e4
    onehot = nc.dram_tensor("onehot", [128, slots], fp8, kind="ExternalInput")
    didx = nc.dram_tensor("didx", [128, slots // 16], i16, kind="ExternalInput")
    score = nc.dram_tensor("score", [128, nwin], f32, kind="ExternalOutput")

    T_tab = nc.dram_tensor("T_tab", [LOC_PAD, 128], bf16)

    with tile.TileContext(nc) as tc:
        nc.gpsimd.load_library(mlp)
        with (
            tc.tile_pool(name="const", bufs=1) as cpool,
            tc.tile_pool(name="loc", bufs=2) as locpool,
            tc.tile_pool(name="psD", bufs=2, space="PSUM") as psD,
            tc.tile_pool(name="ht", bufs=8) as htpool,
            tc.tile_pool(name="win", bufs=4) as winpool,
            tc.tile_pool(name="oh", bufs=8) as ohpool,
            tc.tile_pool(name="ps2", bufs=3, space="PSUM") as ps2pool,
            tc.tile_pool(name="gat", bufs=12) as gpool,
            tc.tile_pool(name="ed", bufs=2) as epool,
            tc.tile_pool(name="idx", bufs=1) as ipool,
            tc.tile_pool(name="out", bufs=1) as opool,
        ):
            wt_t = cpool.tile([128, 128], bf16)
            nc.sync.dma_start(wt_t[:], WT[:])
            m2_t = cpool.tile([128, 128], bf16)
            nc.sync.dma_start(m2_t[:], M2[:])
            gm_t = cpool.tile([128, 128], bf16)
            nc.sync.dma_start(gm_t[:], gamma_bc[:])
            negI_t = cpool.tile([128, 128], bf16)
            nc.sync.dma_start(negI_t[:], negI[:])

            score_sb = opool.tile([128, nwin], f32)

            di_sb = ipool.tile([128, slots // 16], i16)
            nc.sync.dma_start(di_sb[:], didx[:])

            # ---------- phase 1: local T table (this core's dst range) -------
            for fl in range(LOC_TILES // S_FLUSH):
                htl = locpool.tile([128, S_FLUSH * 128], bf16, tag="htl")
                nc.sync.dma_start(
                    htl[:], hT_loc[:, fl * S_FLUSH * 128:(fl + 1) * S_FLUSH * 128])
                stH = locpool.tile([128, S_FLUSH, 128], bf16, tag="stH")
                i = 0
                while i < S_FLUSH:
                    w = min(4, S_FLUSH - i)
                    psT = psD.tile([128, 4, 128], f32, tag="psT")
                    for u in range(w):
                        nc.tensor.matmul(
                            psT[:, u, :],
                            lhsT=htl[:, (i + u) * 128:(i + u + 1) * 128],
                            rhs=m2_t[:])
                    nc.vector.tensor_tensor(
                        out=stH[:, i:i + w, :], in0=psT[:, :w, :],
                        in1=gm_t[:].rearrange("p (a f) -> p a f", a=1)
                        .broadcast_to([128, w, 128]),
                        op=mybir.AluOpType.add,
                    )
                    i += w
                t_write = nc.sync.dma_start(
                    T_tab[fl * S_FLUSH * 128:(fl + 1) * S_FLUSH * 128, :]
                    .rearrange("(p i) f -> p i f", p=128),
                    stH[:],
                )

            # ---------- phase 2: dst gathers (the critical SWDGE pipe) ------
            ps1_ctx = tc.tile_pool(name="ps1", bufs=2, space="PSUM")
            ps2_ctx = tc.tile_pool(name="ps2", bufs=2, space="PSUM")
            ps1pool = ps1_ctx.__enter__()
            ps2pool = ps2_ctx.__enter__()
            gd_tiles = []
            for g in range(NGATH):
                gd = gpool.tile([128, nseg, 128], bf16, tag="gd")
                nc.gpsimd.dma_gather(
                    gd[:], T_tab[:], di_sb[:, g * (NI // 16):(g + 1) * (NI // 16)],
                    NI, NI, 128, single_packet=False, queue_num=g % 4,
                )
                gd_tiles.append(gd)

            # ---------- phase 3: src windows + edge math ---------------------
            diff_tiles = {}
            for wb in range(nwin // WB):
                w0 = wb * WB
                if wb % 2 == 0:
                    hts2 = htpool.tile([128, 2 * WB * 128], bf16, tag="hts")
                    d1 = nc.sync.dma_start(
                        hts2[:], hT_c[:, w0 * 128:(w0 + 2 * WB) * 128])
                    oh2 = ohpool.tile([128, 2 * WB * 128], fp8, tag="oh")
                    d2 = nc.sync.dma_start(
                        oh2[:], onehot[:, w0 * 128:(w0 + 2 * WB) * 128])
                    if wb < 6:
                        # keep phase-1 (T table) DMA unstarved
                        add_dep_helper(d1.ins, t_write.ins,
                                       reason="phase3 streams after T table")
                        add_dep_helper(d2.ins, t_write.ins,
                                       reason="phase3 streams after T table")
                half = (wb % 2) * WB
                ps1 = ps1pool.tile([128, WB, 128], f32, tag="ps1")
                for u in range(WB):
                    nc.tensor.matmul(
                        ps1[:, u, :],
                        lhsT=hts2[:, (half + u) * 128:(half + u + 1) * 128],
                        rhs=wt_t[:],
                    )
                win = winpool.tile([128, WB, 128], bf16, tag="win")
                nc.scalar.activation(
                    win[:], ps1[:], func=mybir.ActivationFunctionType.Copy)

                ps2 = ps2pool.tile([128, WB, 128], f32, tag="ps2")
                for u in range(WB):
                    nc.tensor.matmul(
                        ps2[:, u, :],
                        lhsT=oh2[:, (half + u) * 128:(half + u + 1) * 128],
                        rhs=win[:, u, :],
                    )
                g = w0 // nseg
                s0 = w0 % nseg
                assert s0 + WB <= nseg
                if s0 == 0:
                    diff_w = epool.tile([128, nseg, 128], bf16, tag="diff")
                    diff_tiles[g] = diff_w
                diff_w = diff_tiles[g]
                nc.vector.tensor_tensor(
                    out=diff_w[:, s0:s0 + WB, :],
                    in0=gd_tiles[g][:, s0:s0 + WB, :], in1=ps2[:],
                    op=mybir.AluOpType.subtract,
                )
                if s0 + WB == nseg:
                    sq_w = epool.tile([128, nseg, 128], bf16, tag="sq")
                    if g % 4 != 3:
                        # plain ACT Square at copy rate (no accum read)
                        nc.scalar.activation(
                            sq_w[:], diff_w[:],
                            func=mybir.ActivationFunctionType.Square)
                    else:
                        nc.vector.tensor_tensor(
                            out=sq_w[:], in0=diff_w[:], in1=diff_w[:],
                            op=mybir.AluOpType.mult,
                        )
                    nc.vector.tensor_reduce(
                        out=score_sb[:, g * nseg:(g + 1) * nseg],
                        in_=sq_w[:],
                        axis=mybir.AxisListType.X,
                        op=mybir.AluOpType.add,
                    )

            nc.sync.dma_start(score[:], score_sb[:])
            ps2_ctx.__exit__(None, None, None)
            ps1_ctx.__exit__(None, None, None)

    nc.finalize()
    return nc


def _binpack(ku):
    """Next-fit decreasing: pack rows (ku[r] edges each) into windows with
    <=128 edges and <=128 rows. Returns (win_of_row, pos_of_row, nw)."""
    nu = len(ku)
    order = np.argsort(-ku, kind="stable")
    win_of_row = np.empty(nu, np.int64)
    pos_of_row = np.empty(nu, np.int64)
    w = 0
    edges_w = 0
    rows_w = 0
    for r in order:
        k = int(ku[r])
        if edges_w + k > 128 or rows_w >= 128:
            w += 1
            edges_w = 0
            rows_w = 0
        win_of_row[r] = w
        pos_of_row[r] = rows_w
        edges_w += k
        rows_w += 1
    return win_of_row, pos_of_row, w + 1


def _prep_inputs(h, src, dst, W_w, W_b, H):
    """Host-side: shard edges by dst owner, bin-pack src windows, build
    per-core input maps + slot->edge maps."""
    h = np.asarray(h, dtype=np.float32)
    src = np.asarray(src).astype(np.int64)
    dst = np.asarray(dst).astype(np.int64)
    W_w = np.asarray(W_w, dtype=np.float32)
    W_b = np.asarray(W_b, dtype=np.float32)
    H = np.asarray(H, dtype=np.float32)

    hT_full = h.T.astype(BF16)                              # [128, N]
    WT = np.ascontiguousarray(W_w.T).astype(BF16)           # S: lhsT=hT, rhs=WT
    M2 = (W_w.T @ H).astype(BF16)                           # T: lhsT=hT, rhs=M2
    gamma = (W_b @ H - W_b).astype(np.float32)
    gamma_bc = np.tile(gamma[None, :], (128, 1)).astype(BF16)
    negI_m = (-np.eye(128, dtype=np.float32)).astype(BF16)
    iota128 = np.arange(128)

    owner = np.minimum(dst // OWN, P_CORES - 1)

    in_maps = []
    slotmaps = []
    for c in range(P_CORES):
        sel = np.nonzero(owner == c)[0]
        ne = len(sel)
        if ne > SLOTS:
            raise RuntimeError(f"core {c}: {ne} edges > {SLOTS} slots")
        src_c = src[sel]
        dst_loc = dst[sel] - c * OWN

        uniq, inv = np.unique(src_c, return_inverse=True)
        ku = np.bincount(inv)
        win_of_row, pos_of_row, nw = _binpack(ku)
        if nw > NWIN:
            raise RuntimeError(f"core {c}: {nw} windows > {NWIN}")

        # slot assignment: edges of window w -> slots [128w, 128w + n_w)
        e_win = win_of_row[inv]
        counts = np.bincount(e_win, minlength=nw)
        if counts.max() > 128:
            raise RuntimeError(f"core {c}: window overflow {counts.max()}")
        order_e = np.argsort(e_win, kind="stable")
        starts = np.concatenate([[0], np.cumsum(counts)])
        slot_ids = np.empty(ne, np.int64)
        for w in range(nw):
            lo, hi = starts[w], starts[w + 1]
            slot_ids[order_e[lo:hi]] = 128 * w + np.arange(hi - lo)

        didx_full = np.zeros(SLOTS, np.int64)
        rvals_full = np.full(SLOTS, -1, np.int64)
        smap = np.full(SLOTS, -1, np.int64)
        didx_full[slot_ids] = _sigma(dst_loc)
        rvals_full[slot_ids] = pos_of_row[inv]
        smap[slot_ids] = sel
        onehot_full = (iota128[:, None] == rvals_full[None, :]).astype(
            ml_dtypes.float8_e4m3)

        # packed hT_c: (window w, pos p) -> uniq node column
        hT_win = np.zeros((128, NWIN * 128), dtype=BF16)
        cols = win_of_row * 128 + pos_of_row
        hT_win[:, cols] = hT_full[:, uniq]

        lo = c * OWN
        hT_loc = np.zeros((128, LOC_PAD), dtype=BF16)
        avail = min(N, lo + LOC_PAD) - lo
        hT_loc[:, :avail] = hT_full[:, lo:lo + avail]

        in_maps.append({
            "hT_c": hT_win,
            "hT_loc": hT_loc,
            "WT": WT,
            "M2": M2,
            "gamma_bc": gamma_bc,
            "negI": negI_m,
            "onehot": onehot_full,
            "didx": _pack16r(didx_full, SLOTS),
        })
        slotmaps.append(smap)
    return in_maps, slotmaps


def kernel(h, src, dst, W_w, W_b, H):
    from concourse.bass_utils import run_bass_kernel_spmd

    if "nc" not in _PROG:
        _PROG["nc"] = _build_program()
    nc = _PROG["nc"]

    in_maps, slotmaps = _prep_inputs(h, src, dst, W_w, W_b, H)
    res = run_bass_kernel_spmd(nc, in_maps, list(range(P_CORES)))

    out = np.zeros(E, np.float32)
    for c in range(P_CORES):
        dev = res.results[c]["score"]                       # [128, NWIN]
        padded = dev.T.ravel()                              # slot = 128*w + p
        smap = slotmaps[c]
        m = smap >= 0
        out[smap[m]] = padded[m]
    return out


# revision 7
# speedup vs baseline: 1.0360x; 1.0249x over previous
"""Distributed Trainium2 kernel v2 for nn_DistPredictor (gnn_message_passing).

score[e] = || hp[src[e]] - hpH[dst[e]] ||^2
  hp  = h @ W_w.T + W_b ;  hpH = hp @ H

Math rewrite (bias folded into dst table):
  diff_e = S[src_e] - T[dst_e]
  S[u] = h_u @ W_w.T              (no bias)
  T[v] = h_v @ (W_w.T @ H) + gamma,  gamma = W_b @ H - W_b

Strategy (8 NeuronCores, SPMD single program):
  - Edge -> core by dst ownership (12500-node ranges).
  - dst side: T table for the LOCAL node range only (12544 rows) built in
    DRAM (98 matmuls + sigma-staged row-major writes); per-edge 256B
    dma_gather (the only SWDGE traffic: ~77k descriptors at ~2.5ns/desc,
    the critical path).
  - src side: NO gather. Host bin-packs unique src nodes into NWIN
    windows with <=128 EDGES per window. Device streams packed hT_c,
    one matmul per window builds S rows (node-major, JIT in SBUF), then
    a one-hot matmul (lhsT = iota==rval compare on DVE) replicates
    window rows into edge-major slots.
  - Per window (== one 128-slot gather segment):
    diff = MM2_psum - gathered_T; DVE square; DVE reduce -> score col.
  - Host reassembles scores via slot->edge map.
"""

import sys

if "/opt/trn_rl_repo" not in sys.path:
    sys.path.insert(0, "/opt/trn_rl_repo")

import numpy as np
import ml_dtypes

# ---------------- configuration (patchable for mini-sim) ----------------
D = 128
P_CORES = 8

N = 100000
E = 600000

OWN = N // P_CORES        # nodes owned per core (dst ranges)
S_FLUSH = 7               # node tiles per staging flush for dst table
LOC_TILES = 98            # 98*128 = 12544 padded local nodes
LOC_PAD = LOC_TILES * 128

NI = 2048                 # indices per dma_gather instruction
NGATH = 38                # gather instructions per core
SLOTS = NI * NGATH        # 77824 edge slots = NWIN*128
NWIN = SLOTS // 128       # 608 src windows
WB = 8                    # windows per psum/staging batch

BF16 = ml_dtypes.bfloat16

_PROG = {}


def _sigma(local_node):
    """chunk-local node id -> permuted table row (matches staging flush)."""
    t = local_node // 128
    p = local_node % 128
    g = t // S_FLUSH
    i = t % S_FLUSH
    return g * (128 * S_FLUSH) + p * S_FLUSH + i


def _pack16r(idx, cap):
    """Pack idx -> [128, cap//16] int16 (i -> [i%16, i//16]), replicated
    across the 8 16-partition groups."""
    s = cap // 16
    out = np.zeros((16, s), np.int16)
    ar = np.arange(cap)
    out[ar % 16, ar // 16] = idx.astype(np.int16)
    return np.tile(out, (8, 1))


def _patch_swdge_lane_pinning():
    """Pin DMASW sem lanes {2q, 2q+1} to SWDGE queue q (Tile's round-robin
    is queue-unaware; with >1 queues a lane can see two queues' completions,
    breaking Tile's FIFO wait assumption)."""
    from concourse import tile_sem_assignment as tsa
    from concourse import mybir
    from concourse.tile_scheduler import DMAInst

    if getattr(tsa, "_qpin_patched", False):
        return
    cls = tsa.TileClockTick
    orig = cls._assign_tick

    def patched(self, inst):
        qn = getattr(inst, "queue_num", None)
        if (
            isinstance(inst, DMAInst)
            and inst.engine == mybir.EngineType.Pool
            and qn is not None
        ):
            lane_map = self.__dict__.setdefault("_qpin_map", {})
            if inst.name not in lane_map:
                cnts = self.__dict__.setdefault("_qpin_cnt", {})
                c = cnts.get(qn, 0)
                lane_map[inst.name] = (2 * qn + (c % 2)) % 8
                cnts[qn] = c + 1
            self.next_sw_dma_idx = lane_map[inst.name]
        return orig(self, inst)

    cls._assign_tick = patched
    tsa._qpin_patched = True


def _build_program():
    import concourse.bass as bass
    import concourse.tile as tile
    from concourse import bacc, mybir
    from concourse.library_config import mlp
    from concourse.tile_rust import add_dep_helper

    _patch_swdge_lane_pinning()

    f32 = mybir.dt.float32
    bf16 = mybir.dt.bfloat16
    i16 = mybir.dt.int16

    nc = bacc.Bacc(
        "TRN2",
        target_bir_lowering=False,
        debug=False,
        num_devices=P_CORES,
        num_swdge_queues=4,
    )

    slots = SLOTS
    nwin = NWIN
    nseg = NI // 128

    hT_c = nc.dram_tensor("hT_c", [128, nwin * 128], bf16, kind="ExternalInput")
    hT_loc = nc.dram_tensor("hT_loc", [128, LOC_PAD], bf16, kind="ExternalInput")
    WT = nc.dram_tensor("WT", [128, 128], bf16, kind="ExternalInput")
    M2 = nc.dram_tensor("M2", [128, 128], bf16, kind="ExternalInput")
    gamma_bc = nc.dram_tensor("gamma_bc", [128, 128], bf16, kind="ExternalInput")
    negI = nc.dram_tensor("negI", [128, 128], bf16, kind="ExternalInput")
    fp8 = mybir.dt.float8e4
    onehot = nc.dram_tensor("onehot", [128, slots], fp8, kind="ExternalInput")
    didx = nc.dram_tensor("didx", [128, slots // 16], i16, kind="ExternalInput")
    score = nc.dram_tensor("score", [128, nwin], f32, kind="ExternalOutput")

    T_tab = nc.dram_tensor("T_tab", [LOC_PAD, 128], bf16)

    with tile.TileContext(nc) as tc:
        nc.gpsimd.load_library(mlp)
        with (
            tc.tile_pool(name="const", bufs=1) as cpool,
            tc.tile_pool(name="loc", bufs=2) as locpool,
            tc.tile_pool(name="psD", bufs=2, space="PSUM") as psD,
            tc.tile_pool(name="ht", bufs=8) as htpool,
            tc.tile_pool(name="win", bufs=4) as winpool,
            tc.tile_pool(name="oh", bufs=8) as ohpool,
            tc.tile_pool(name="ps2", bufs=3, space="PSUM") as ps2pool,
            tc.tile_pool(name="gat", bufs=12) as gpool,
            tc.tile_pool(name="ed", bufs=2) as epool,
            tc.tile_pool(name="idx", bufs=1) as ipool,
            tc.tile_pool(name="out", bufs=1) as opool,
        ):
            wt_t = cpool.tile([128, 128], bf16)
            nc.sync.dma_start(wt_t[:], WT[:])
            m2_t = cpool.tile([128, 128], bf16)
            nc.sync.dma_start(m2_t[:], M2[:])
            gm_t = cpool.tile([128, 128], bf16)
            nc.sync.dma_start(gm_t[:], gamma_bc[:])
            negI_t = cpool.tile([128, 128], bf16)
            nc.sync.dma_start(negI_t[:], negI[:])

            score_sb = opool.tile([128, nwin], f32)

            di_sb = ipool.tile([128, slots // 16], i16)
            nc.sync.dma_start(di_sb[:], didx[:])

            # ---------- phase 1: local T table (this core's dst range) -------
            for fl in range(LOC_TILES // S_FLUSH):
                htl = locpool.tile([128, S_FLUSH * 128], bf16, tag="htl")
                nc.sync.dma_start(
                    htl[:], hT_loc[:, fl * S_FLUSH * 128:(fl + 1) * S_FLUSH * 128])
                stH = locpool.tile([128, S_FLUSH, 128], bf16, tag="stH")
                i = 0
                while i < S_FLUSH:
                    w = min(4, S_FLUSH - i)
                    psT = psD.tile([128, 4, 128], f32, tag="psT")
                    for u in range(w):
                        nc.tensor.matmul(
                            psT[:, u, :],
                            lhsT=htl[:, (i + u) * 128:(i + u + 1) * 128],
                            rhs=m2_t[:])
                    nc.vector.tensor_tensor(
                        out=stH[:, i:i + w, :], in0=psT[:, :w, :],
                        in1=gm_t[:].rearrange("p (a f) -> p a f", a=1)
                        .broadcast_to([128, w, 128]),
                        op=mybir.AluOpType.add,
                    )
                    i += w
                t_write = nc.sync.dma_start(
                    T_tab[fl * S_FLUSH * 128:(fl + 1) * S_FLUSH * 128, :]
                    .rearrange("(p i) f -> p i f", p=128),
                    stH[:],
                )

            # ---------- phase 2: dst gathers (the critical SWDGE pipe) ------
            ps1_ctx = tc.tile_pool(name="ps1", bufs=2, space="PSUM")
            ps2_ctx = tc.tile_pool(name="ps2", bufs=2, space="PSUM")
            ps1pool = ps1_ctx.__enter__()
            ps2pool = ps2_ctx.__enter__()
            gd_tiles = []
            for g in range(NGATH):
                gd = gpool.tile([128, nseg, 128], bf16, tag="gd")
                nc.gpsimd.dma_gather(
                    gd[:], T_tab[:], di_sb[:, g * (NI // 16):(g + 1) * (NI // 16)],
                    NI, NI, 128, single_packet=False, queue_num=g % 4,
                )
                gd_tiles.append(gd)

            # ---------- phase 3: src windows + edge math ---------------------
            diff_tiles = {}
            for wb in range(nwin // WB):
                w0 = wb * WB
                if wb % 2 == 0:
                    hts2 = htpool.tile([128, 2 * WB * 128], bf16, tag="hts")
                    d1 = nc.sync.dma_start(
                        hts2[:], hT_c[:, w0 * 128:(w0 + 2 * WB) * 128])
                    oh2 = ohpool.tile([128, 2 * WB * 128], fp8, tag="oh")
                    d2 = nc.sync.dma_start(
                        oh2[:], onehot[:, w0 * 128:(w0 + 2 * WB) * 128])
                    if wb < 6:
                        # keep phase-1 (T table) DMA unstarved
                        add_dep_helper(d1.ins, t_write.ins,
                                       reason="phase3 streams after T table")
                        add_dep_helper(d2.ins, t_write.ins,
                                       reason="phase3 streams after T table")
                half = (wb % 2) * WB
                ps1 = ps1pool.tile([128, WB, 128], f32, tag="ps1")
                for u in range(WB):
                    nc.tensor.matmul(
                        ps1[:, u, :],
                        lhsT=hts2[:, (half + u) * 128:(half + u + 1) * 128],
                        rhs=wt_t[:],
                    )
                win = winpool.tile([128, WB, 128], bf16, tag="win")
                nc.scalar.activation(
                    win[:], ps1[:], func=mybir.ActivationFunctionType.Copy)

                ps2 = ps2pool.tile([128, WB, 128], f32, tag="ps2")
                for u in range(WB):
                    nc.tensor.matmul(
                        ps2[:, u, :],
                        lhsT=oh2[:, (half + u) * 128:(half + u + 1) * 128],
                        rhs=win[:, u, :],
                    )
                g = w0 // nseg
                s0 = w0 % nseg
                assert s0 + WB <= nseg
                if s0 == 0:
                    diff_w = epool.tile([128, nseg, 128], bf16, tag="diff")
                    diff_tiles[g] = diff_w
                diff_w = diff_tiles[g]
                nc.vector.tensor_tensor(
                    out=diff_w[:, s0:s0 + WB, :],
                    in0=gd_tiles[g][:, s0:s0 + WB, :], in1=ps2[:],
                    op=mybir.AluOpType.subtract,
                )
                if s0 + WB == nseg:
                    sq_w = epool.tile([128, nseg, 128], bf16, tag="sq")
                    if g % 4 != 3:
                        # plain ACT Square at copy rate (no accum read)
                        nc.scalar.activation(
                            sq_w[:], diff_w[:],
                            func=mybir.ActivationFunctionType.Square)
                    else:
                        nc.vector.tensor_tensor(
                            out=sq_w[:], in0=diff_w[:], in1=diff_w[:],
                            op=mybir.AluOpType.mult,
                        )
                    nc.vector.tensor_reduce(
                        out=score_sb[:, g * nseg:(g + 1) * nseg],
                        in_=sq_w[:],
                        axis=mybir.AxisListType.X,
                        op=mybir.AluOpType.add,
                    )

            nc.sync.dma_start(score[:], score_sb[:])
            ps2_ctx.__exit__(None, None, None)
            ps1_ctx.__exit__(None, None, None)

    nc.finalize()
    return nc


def _binpack(ku):
    """Next-fit decreasing: pack rows (ku[r] edges each) into windows with
    <=128 edges and <=128 rows. Returns (win_of_row, pos_of_row, nw)."""
    nu = len(ku)
    order = np.argsort(-ku, kind="stable")
    win_of_row = np.empty(nu, np.int64)
    pos_of_row = np.empty(nu, np.int64)
    w = 0
    edges_w = 0
    rows_w = 0
    for r in order:
        k = int(ku[r])
        if edges_w + k > 128 or rows_w >= 128:
            w += 1
            edges_w = 0
            rows_w = 0
        win_of_row[r] = w
        pos_of_row[r] = rows_w
        edges_w += k
        rows_w += 1
    return win_of_row, pos_of_row, w + 1


def _prep_inputs(h, src, dst, W_w, W_b, H):
    """Host-side: shard edges by dst owner, bin-pack src windows, build
    per-core input maps + slot->edge maps."""
    h = np.asarray(h, dtype=np.float32)
    src = np.asarray(src).astype(np.int64)
    dst = np.asarray(dst).astype(np.int64)
    W_w = np.asarray(W_w, dtype=np.float32)
    W_b = np.asarray(W_b, dtype=np.float32)
    H = np.asarray(H, dtype=np.float32)

    hT_full = h.T.astype(BF16)                              # [128, N]
    WT = np.ascontiguousarray(W_w.T).astype(BF16)           # S: lhsT=hT, rhs=WT
    M2 = (W_w.T @ H).astype(BF16)                           # T: lhsT=hT, rhs=M2
    gamma = (W_b @ H - W_b).astype(np.float32)
    gamma_bc = np.tile(gamma[None, :], (128, 1)).astype(BF16)
    negI_m = (-np.eye(128, dtype=np.float32)).astype(BF16)
    iota128 = np.arange(128)

    owner = np.minimum(dst // OWN, P_CORES - 1)

    in_maps = []
    slotmaps = []
    for c in range(P_CORES):
        sel = np.nonzero(owner == c)[0]
        ne = len(sel)
        if ne > SLOTS:
            raise RuntimeError(f"core {c}: {ne} edges > {SLOTS} slots")
        src_c = src[sel]
        dst_loc = dst[sel] - c * OWN

        uniq, inv = np.unique(src_c, return_inverse=True)
        ku = np.bincount(inv)
        win_of_row, pos_of_row, nw = _binpack(ku)
        if nw > NWIN:
            raise RuntimeError(f"core {c}: {nw} windows > {NWIN}")

        # slot assignment: edges of window w -> slots [128w, 128w + n_w)
        e_win = win_of_row[inv]
        counts = np.bincount(e_win, minlength=nw)
        if counts.max() > 128:
            raise RuntimeError(f"core {c}: window overflow {counts.max()}")
        order_e = np.argsort(e_win, kind="stable")
        starts = np.concatenate([[0], np.cumsum(counts)])
        slot_ids = np.empty(ne, np.int64)
        for w in range(nw):
            lo, hi = starts[w], starts[w + 1]
            slot_ids[order_e[lo:hi]] = 128 * w + np.arange(hi - lo)

        didx_full = np.zeros(SLOTS, np.int64)
        rvals_full = np.full(SLOTS, -1, np.int64)
        smap = np.full(SLOTS, -1, np.int64)
        didx_full[slot_ids] = _sigma(dst_loc)
        rvals_full[slot_ids] = pos_of_row[inv]
        smap[slot_ids] = sel
        onehot_full = (iota128[:, None] == rvals_full[None, :]).astype(
            ml_dtypes.float8_e4m3)

        # packed hT_c: (window w, pos p) -> uniq node column
        hT_win = np.zeros((128, NWIN * 128), dtype=BF16)
        cols = win_of_row * 128 + pos_of_row
        hT_win[:, cols] = hT_full[:, uniq]

        lo = c * OWN
        hT_loc = np.zeros((128, LOC_PAD), dtype=BF16)
        avail = min(N, lo + LOC_PAD) - lo
        hT_loc[:, :avail] = hT_full[:, lo:lo + avail]

        in_maps.append({
            "hT_c": hT_win,
            "hT_loc": hT_loc,
            "WT": WT,
            "M2": M2,
            "gamma_bc": gamma_bc,
            "negI": negI_m,
            "onehot": onehot_full,
            "didx": _pack16r(didx_full, SLOTS),
        })
        slotmaps.append(smap)
    return in_maps, slotmaps


def kernel(h, src, dst, W_w, W_b, H):
    from concourse.bass_utils import run_bass_kernel_spmd

    if "nc" not in _PROG:
        _PROG["nc"] = _build_program()
    nc = _PROG["nc"]

    in_maps, slotmaps = _prep_inputs(h, src, dst, W_w, W_b, H)
    res = run_bass_kernel_spmd(nc, in_maps, list(range(P_CORES)))

    out = np.zeros(E, np.float32)
    for c in range(P_CORES):
        dev = res.results[c]["score"]                       # [128, NWIN]
        padded = dev.T.ravel()                              # slot = 128*w + p
        smap = slotmaps[c]
        m = smap >= 0
        out[smap[m]] = padded[m]
    return out


# revision 8
# speedup vs baseline: 1.1949x; 1.1534x over previous
"""Distributed Trainium2 kernel v2 for nn_DistPredictor (gnn_message_passing).

score[e] = || hp[src[e]] - hpH[dst[e]] ||^2
  hp  = h @ W_w.T + W_b ;  hpH = hp @ H

Math rewrite (bias folded into dst table):
  diff_e = S[src_e] - T[dst_e]
  S[u] = h_u @ W_w.T              (no bias)
  T[v] = h_v @ (W_w.T @ H) + gamma,  gamma = W_b @ H - W_b

Strategy (8 NeuronCores, SPMD single program):
  - Edge -> core by dst ownership (12500-node ranges).
  - dst side: T table for the LOCAL node range only (12544 rows) built in
    DRAM (98 matmuls + sigma-staged row-major writes); per-edge 256B
    dma_gather (the only SWDGE traffic: ~77k descriptors at ~2.5ns/desc,
    the critical path).
  - src side: NO gather. Host bin-packs unique src nodes into NWIN
    windows with <=128 EDGES per window. Device streams packed hT_c,
    one matmul per window builds S rows (node-major, JIT in SBUF), then
    a one-hot matmul (lhsT = iota==rval compare on DVE) replicates
    window rows into edge-major slots.
  - Per window (== one 128-slot gather segment):
    diff = MM2_psum - gathered_T; DVE square; DVE reduce -> score col.
  - Host reassembles scores via slot->edge map.
"""

import sys

if "/opt/trn_rl_repo" not in sys.path:
    sys.path.insert(0, "/opt/trn_rl_repo")

import numpy as np
import ml_dtypes

# ---------------- configuration (patchable for mini-sim) ----------------
D = 128
P_CORES = 8

N = 100000
E = 600000

OWN = N // P_CORES        # nodes owned per core (dst ranges)
S_FLUSH = 7               # node tiles per staging flush for dst table
LOC_TILES = 98            # 98*128 = 12544 padded local nodes
LOC_PAD = LOC_TILES * 128

NI = 2048                 # indices per dma_gather instruction
NGATH = 38                # gather instructions per core
SLOTS = NI * NGATH        # 77824 edge slots = NWIN*128
NWIN = SLOTS // 128       # 608 src windows
WB = 8                    # windows per psum/staging batch

BF16 = ml_dtypes.bfloat16

_PROG = {}


def _sigma(local_node):
    """chunk-local node id -> permuted table row (matches staging flush)."""
    t = local_node // 128
    p = local_node % 128
    g = t // S_FLUSH
    i = t % S_FLUSH
    return g * (128 * S_FLUSH) + p * S_FLUSH + i


def _pack16r(idx, cap):
    """Pack idx -> [128, cap//16] int16 (i -> [i%16, i//16]), replicated
    across the 8 16-partition groups."""
    s = cap // 16
    out = np.zeros((16, s), np.int16)
    ar = np.arange(cap)
    out[ar % 16, ar // 16] = idx.astype(np.int16)
    return np.tile(out, (8, 1))


def _patch_swdge_lane_pinning():
    """Pin DMASW sem lanes {2q, 2q+1} to SWDGE queue q (Tile's round-robin
    is queue-unaware; with >1 queues a lane can see two queues' completions,
    breaking Tile's FIFO wait assumption)."""
    from concourse import tile_sem_assignment as tsa
    from concourse import mybir
    from concourse.tile_scheduler import DMAInst

    if getattr(tsa, "_qpin_patched", False):
        return
    cls = tsa.TileClockTick
    orig = cls._assign_tick

    def patched(self, inst):
        qn = getattr(inst, "queue_num", None)
        if (
            isinstance(inst, DMAInst)
            and inst.engine == mybir.EngineType.Pool
            and qn is not None
        ):
            lane_map = self.__dict__.setdefault("_qpin_map", {})
            if inst.name not in lane_map:
                cnts = self.__dict__.setdefault("_qpin_cnt", {})
                c = cnts.get(qn, 0)
                lane_map[inst.name] = (2 * qn + (c % 2)) % 8
                cnts[qn] = c + 1
            self.next_sw_dma_idx = lane_map[inst.name]
        return orig(self, inst)

    cls._assign_tick = patched
    tsa._qpin_patched = True


def _build_program():
    import concourse.bass as bass
    import concourse.tile as tile
    from concourse import bacc, mybir
    from concourse.library_config import mlp
    from concourse.tile_rust import add_dep_helper

    _patch_swdge_lane_pinning()

    f32 = mybir.dt.float32
    bf16 = mybir.dt.bfloat16
    i16 = mybir.dt.int16

    nc = bacc.Bacc(
        "TRN2",
        target_bir_lowering=False,
        debug=False,
        num_devices=P_CORES,
        num_swdge_queues=4,
    )

    slots = SLOTS
    nwin = NWIN
    nseg = NI // 128

    hT_c = nc.dram_tensor("hT_c", [128, nwin * 128], bf16, kind="ExternalInput")
    hT_loc = nc.dram_tensor("hT_loc", [128, LOC_PAD], bf16, kind="ExternalInput")
    WT = nc.dram_tensor("WT", [128, 128], bf16, kind="ExternalInput")
    M2 = nc.dram_tensor("M2", [128, 128], bf16, kind="ExternalInput")
    gamma_bc = nc.dram_tensor("gamma_bc", [128, 128], bf16, kind="ExternalInput")
    negI = nc.dram_tensor("negI", [128, 128], bf16, kind="ExternalInput")
    fp8 = mybir.dt.float8e4
    onehot = nc.dram_tensor("onehot", [128, slots], fp8, kind="ExternalInput")
    didx = nc.dram_tensor("didx", [128, slots // 16], i16, kind="ExternalInput")
    score = nc.dram_tensor("score", [128, nwin], f32, kind="ExternalOutput")

    T_tab = nc.dram_tensor("T_tab", [LOC_PAD, 128], bf16)

    with tile.TileContext(nc) as tc:
        nc.gpsimd.load_library(mlp)
        with (
            tc.tile_pool(name="const", bufs=1) as cpool,
            tc.tile_pool(name="loc", bufs=2) as locpool,
            tc.tile_pool(name="psD", bufs=2, space="PSUM") as psD,
            tc.tile_pool(name="ht", bufs=12) as htpool,
            tc.tile_pool(name="win", bufs=4) as winpool,
            tc.tile_pool(name="oh", bufs=12) as ohpool,
            tc.tile_pool(name="ps2", bufs=3, space="PSUM") as ps2pool,
            tc.tile_pool(name="gat", bufs=16) as gpool,
            tc.tile_pool(name="ed", bufs=2) as epool,
            tc.tile_pool(name="idx", bufs=1) as ipool,
            tc.tile_pool(name="out", bufs=1) as opool,
        ):
            wt_t = cpool.tile([128, 128], bf16)
            nc.sync.dma_start(wt_t[:], WT[:])
            m2_t = cpool.tile([128, 128], bf16)
            nc.sync.dma_start(m2_t[:], M2[:])
            gm_t = cpool.tile([128, 128], bf16)
            nc.sync.dma_start(gm_t[:], gamma_bc[:])
            negI_t = cpool.tile([128, 128], bf16)
            nc.sync.dma_start(negI_t[:], negI[:])

            score_sb = opool.tile([128, nwin], f32)

            di_sb = ipool.tile([128, slots // 16], i16)
            nc.sync.dma_start(di_sb[:], didx[:])

            # ---------- phase 1: local T table (this core's dst range) -------
            for fl in range(LOC_TILES // S_FLUSH):
                htl = locpool.tile([128, S_FLUSH * 128], bf16, tag="htl")
                nc.sync.dma_start(
                    htl[:], hT_loc[:, fl * S_FLUSH * 128:(fl + 1) * S_FLUSH * 128])
                stH = locpool.tile([128, S_FLUSH, 128], bf16, tag="stH")
                i = 0
                while i < S_FLUSH:
                    w = min(4, S_FLUSH - i)
                    psT = psD.tile([128, 4, 128], f32, tag="psT")
                    for u in range(w):
                        nc.tensor.matmul(
                            psT[:, u, :],
                            lhsT=htl[:, (i + u) * 128:(i + u + 1) * 128],
                            rhs=m2_t[:])
                    nc.vector.tensor_tensor(
                        out=stH[:, i:i + w, :], in0=psT[:, :w, :],
                        in1=gm_t[:].rearrange("p (a f) -> p a f", a=1)
                        .broadcast_to([128, w, 128]),
                        op=mybir.AluOpType.add,
                    )
                    i += w
                t_write = nc.sync.dma_start(
                    T_tab[fl * S_FLUSH * 128:(fl + 1) * S_FLUSH * 128, :]
                    .rearrange("(p i) f -> p i f", p=128),
                    stH[:],
                )

            # ---------- phase 2: dst gathers (the critical SWDGE pipe) ------
            ps1_ctx = tc.tile_pool(name="ps1", bufs=2, space="PSUM")
            ps2_ctx = tc.tile_pool(name="ps2", bufs=2, space="PSUM")
            ps1pool = ps1_ctx.__enter__()
            ps2pool = ps2_ctx.__enter__()
            gd_tiles = []
            for g in range(NGATH):
                gd = gpool.tile([128, nseg, 128], bf16, tag="gd")
                nc.gpsimd.dma_gather(
                    gd[:], T_tab[:], di_sb[:, g * (NI // 16):(g + 1) * (NI // 16)],
                    NI, NI, 128, single_packet=False, queue_num=g % 4,
                )
                gd_tiles.append(gd)

            # ---------- phase 3: src windows + edge math ---------------------
            diff_tiles = {}
            for wb in range(nwin // WB):
                w0 = wb * WB
                if wb % 2 == 0:
                    hts2 = htpool.tile([128, 2 * WB * 128], bf16, tag="hts")
                    d1 = nc.sync.dma_start(
                        hts2[:], hT_c[:, w0 * 128:(w0 + 2 * WB) * 128])
                    oh2 = ohpool.tile([128, 2 * WB * 128], fp8, tag="oh")
                    d2 = nc.sync.dma_start(
                        oh2[:], onehot[:, w0 * 128:(w0 + 2 * WB) * 128])
                    if wb < 6:
                        # keep phase-1 (T table) DMA unstarved
                        add_dep_helper(d1.ins, t_write.ins,
                                       reason="phase3 streams after T table")
                        add_dep_helper(d2.ins, t_write.ins,
                                       reason="phase3 streams after T table")
                half = (wb % 2) * WB
                ps1 = ps1pool.tile([128, WB, 128], f32, tag="ps1")
                for u in range(WB):
                    nc.tensor.matmul(
                        ps1[:, u, :],
                        lhsT=hts2[:, (half + u) * 128:(half + u + 1) * 128],
                        rhs=wt_t[:],
                    )
                win = winpool.tile([128, WB, 128], bf16, tag="win")
                nc.scalar.activation(
                    win[:], ps1[:], func=mybir.ActivationFunctionType.Copy)

                ps2 = ps2pool.tile([128, WB, 128], f32, tag="ps2")
                for u in range(WB):
                    nc.tensor.matmul(
                        ps2[:, u, :],
                        lhsT=oh2[:, (half + u) * 128:(half + u + 1) * 128],
                        rhs=win[:, u, :],
                    )
                g = w0 // nseg
                s0 = w0 % nseg
                assert s0 + WB <= nseg
                if s0 == 0:
                    diff_w = epool.tile([128, nseg, 128], bf16, tag="diff")
                    diff_tiles[g] = diff_w
                diff_w = diff_tiles[g]
                nc.vector.tensor_tensor(
                    out=diff_w[:, s0:s0 + WB, :],
                    in0=gd_tiles[g][:, s0:s0 + WB, :], in1=ps2[:],
                    op=mybir.AluOpType.subtract,
                )
                if s0 + WB == nseg:
                    sq_w = epool.tile([128, nseg, 128], bf16, tag="sq")
                    if g % 4 != 3:
                        # plain ACT Square at copy rate (no accum read)
                        nc.scalar.activation(
                            sq_w[:], diff_w[:],
                            func=mybir.ActivationFunctionType.Square)
                    else:
                        nc.vector.tensor_tensor(
                            out=sq_w[:], in0=diff_w[:], in1=diff_w[:],
                            op=mybir.AluOpType.mult,
                        )
                    nc.vector.tensor_reduce(
                        out=score_sb[:, g * nseg:(g + 1) * nseg],
                        in_=sq_w[:],
                        axis=mybir.AxisListType.X,
                        op=mybir.AluOpType.add,
                    )

            nc.sync.dma_start(score[:], score_sb[:])
            ps2_ctx.__exit__(None, None, None)
            ps1_ctx.__exit__(None, None, None)

    nc.finalize()
    return nc


def _binpack(ku):
    """Next-fit decreasing: pack rows (ku[r] edges each) into windows with
    <=128 edges and <=128 rows. Returns (win_of_row, pos_of_row, nw)."""
    nu = len(ku)
    order = np.argsort(-ku, kind="stable")
    win_of_row = np.empty(nu, np.int64)
    pos_of_row = np.empty(nu, np.int64)
    w = 0
    edges_w = 0
    rows_w = 0
    for r in order:
        k = int(ku[r])
        if edges_w + k > 128 or rows_w >= 128:
            w += 1
            edges_w = 0
            rows_w = 0
        win_of_row[r] = w
        pos_of_row[r] = rows_w
        edges_w += k
        rows_w += 1
    return win_of_row, pos_of_row, w + 1


def _prep_inputs(h, src, dst, W_w, W_b, H):
    """Host-side: shard edges by dst owner, bin-pack src windows, build
    per-core input maps + slot->edge maps."""
    h = np.asarray(h, dtype=np.float32)
    src = np.asarray(src).astype(np.int64)
    dst = np.asarray(dst).astype(np.int64)
    W_w = np.asarray(W_w, dtype=np.float32)
    W_b = np.asarray(W_b, dtype=np.float32)
    H = np.asarray(H, dtype=np.float32)

    hT_full = h.T.astype(BF16)                              # [128, N]
    WT = np.ascontiguousarray(W_w.T).astype(BF16)           # S: lhsT=hT, rhs=WT
    M2 = (W_w.T @ H).astype(BF16)                           # T: lhsT=hT, rhs=M2
    gamma = (W_b @ H - W_b).astype(np.float32)
    gamma_bc = np.tile(gamma[None, :], (128, 1)).astype(BF16)
    negI_m = (-np.eye(128, dtype=np.float32)).astype(BF16)
    iota128 = np.arange(128)

    owner = np.minimum(dst // OWN, P_CORES - 1)

    in_maps = []
    slotmaps = []
    for c in range(P_CORES):
        sel = np.nonzero(owner == c)[0]
        ne = len(sel)
        if ne > SLOTS:
            raise RuntimeError(f"core {c}: {ne} edges > {SLOTS} slots")
        src_c = src[sel]
        dst_loc = dst[sel] - c * OWN

        uniq, inv = np.unique(src_c, return_inverse=True)
        ku = np.bincount(inv)
        win_of_row, pos_of_row, nw = _binpack(ku)
        if nw > NWIN:
            raise RuntimeError(f"core {c}: {nw} windows > {NWIN}")

        # slot assignment: edges of window w -> slots [128w, 128w + n_w)
        e_win = win_of_row[inv]
        counts = np.bincount(e_win, minlength=nw)
        if counts.max() > 128:
            raise RuntimeError(f"core {c}: window overflow {counts.max()}")
        order_e = np.argsort(e_win, kind="stable")
        starts = np.concatenate([[0], np.cumsum(counts)])
        slot_ids = np.empty(ne, np.int64)
        for w in range(nw):
            lo, hi = starts[w], starts[w + 1]
            slot_ids[order_e[lo:hi]] = 128 * w + np.arange(hi - lo)

        didx_full = np.zeros(SLOTS, np.int64)
        rvals_full = np.full(SLOTS, -1, np.int64)
        smap = np.full(SLOTS, -1, np.int64)
        didx_full[slot_ids] = _sigma(dst_loc)
        rvals_full[slot_ids] = pos_of_row[inv]
        smap[slot_ids] = sel
        onehot_full = (iota128[:, None] == rvals_full[None, :]).astype(
            ml_dtypes.float8_e4m3)

        # packed hT_c: (window w, pos p) -> uniq node column
        hT_win = np.zeros((128, NWIN * 128), dtype=BF16)
        cols = win_of_row * 128 + pos_of_row
        hT_win[:, cols] = hT_full[:, uniq]

        lo = c * OWN
        hT_loc = np.zeros((128, LOC_PAD), dtype=BF16)
        avail = min(N, lo + LOC_PAD) - lo
        hT_loc[:, :avail] = hT_full[:, lo:lo + avail]

        in_maps.append({
            "hT_c": hT_win,
            "hT_loc": hT_loc,
            "WT": WT,
            "M2": M2,
            "gamma_bc": gamma_bc,
            "negI": negI_m,
            "onehot": onehot_full,
            "didx": _pack16r(didx_full, SLOTS),
        })
        slotmaps.append(smap)
    return in_maps, slotmaps


def kernel(h, src, dst, W_w, W_b, H):
    from concourse.bass_utils import run_bass_kernel_spmd

    if "nc" not in _PROG:
        _PROG["nc"] = _build_program()
    nc = _PROG["nc"]

    in_maps, slotmaps = _prep_inputs(h, src, dst, W_w, W_b, H)
    res = run_bass_kernel_spmd(nc, in_maps, list(range(P_CORES)))

    out = np.zeros(E, np.float32)
    for c in range(P_CORES):
        dev = res.results[c]["score"]                       # [128, NWIN]
        padded = dev.T.ravel()                              # slot = 128*w + p
        smap = slotmaps[c]
        m = smap >= 0
        out[smap[m]] = padded[m]
    return out
